# revision 1
# baseline (speedup 1.0000x reference)
"""Bass/Trainium2 kernel for ExtractPatchesPosition (bilinear patch extraction).

Strategy (pure data parallel, batch sharded over 8 cores; 256 samples/core):

For each (sample b, channel c) the reference samples a translated N x N grid
out(r,col) = img(r + 32 + oy, col + 32 + ox) with bilinear interpolation.
With |offset| <= 20 and margin 32 the samples never leave the image, so the
whole patch is: take the (N+1) x (N+1) window at integer origin
(y0, x0) = (floor(32+oy), floor(32+ox)) and blend

    t = (1-fy)*W[r, x]   + fy*W[r+1, x]      (vertical 2-tap)
    o = (1-fx)*t[r, col] + fx*t[r, col+1]    (horizontal 2-tap)

Device pipeline, per group of 32 samples (= 128 (b,c) windows on 128
partitions):
  1. one indirect DMA (SWDGE) gathers, per partition, a contiguous run of
     65*128 elements of the flat image starting at the window origin
     (b*128 + y0)*128 + x0.  Both data-dependent shifts are absorbed into the
     per-partition element-granularity start offset; inside the run the
     window sits at static offsets (r*128 + x).
  2. ACT (scale per partition) + DVE scalar_tensor_tensor do the two fused
     blends on strided views -> o[128 windows, 64*64].
  3. PE transpose_mode flips each 128-col chunk so rows/cols land on
     partitions and windows (b,c) land on the free dim in channel-interleaved
     order; ACT/DVE copy PSUM->SBUF.
  4. two HWDGE DMAs per group write the (b, r, col, ch) output with
     contiguous 1 KiB rows per sample on the HBM side.

The tiny per-window metadata (int window origins, fractional weights) is
precomputed on host from `positions` (O(B*C) work) and passed as extra
input tensors; all O(B*N*N*C) data movement and math runs on device.
"""

import numpy as np

import concourse.bacc as bacc
import concourse.tile as tile
from concourse import mybir
from concourse.bass import IndirectOffsetOnAxis

B, M, N, C = 2048, 128, 64, 4
NCORES = 8
BC = B // NCORES          # 256 samples per core
GSAMP = 32                # samples per group
GROUPS = BC // GSAMP      # 8 groups
NW = GSAMP * C            # 128 windows per group (one per partition)
ROWS = N + 1              # 65 window rows
RUN = ROWS * M            # 8320 gathered elements per window
F32 = mybir.dt.float32
Copy = mybir.ActivationFunctionType.Copy
MULT = mybir.AluOpType.mult
ADD = mybir.AluOpType.add

_NC_CACHE = {}


def _build_nc():
    nc = bacc.Bacc("TRN2")
    img = nc.declare_dram_parameter("img", [BC * M * M, 1], F32, isOutput=False)
    idx = nc.declare_dram_parameter("idx", [128, GROUPS], mybir.dt.int32, isOutput=False)
    meta = nc.declare_dram_parameter("meta", [128, 4 * GROUPS], F32, isOutput=False)
    ident = nc.declare_dram_parameter("ident", [128, 128], F32, isOutput=False)
    out = nc.declare_dram_parameter("out", [BC, N, N, C], F32, isOutput=True)

    with tile.TileContext(nc) as tc:
        with (
            tc.tile_pool(name="singles", bufs=1) as singles,
            tc.tile_pool(name="gpool", bufs=2) as gpool,
            tc.tile_pool(name="tpool", bufs=1) as tpool,
            tc.tile_pool(name="uvpool", bufs=1) as uvpool,
            tc.tile_pool(name="opool", bufs=2) as opool,
            tc.tile_pool(name="obpool", bufs=2) as obpool,
            tc.tile_pool(name="psum", bufs=4, space="PSUM") as psump,
        ):
            idx_sb = singles.tile([128, GROUPS], mybir.dt.int32)
            meta_sb = singles.tile([128, 4 * GROUPS], F32)
            ident_sb = singles.tile([128, 128], F32)
            nc.sync.dma_start(idx_sb[:], idx[:])
            nc.sync.dma_start(meta_sb[:], meta[:])
            nc.sync.dma_start(ident_sb[:], ident[:])

            # out[b, r, c, h]: walk order (k, p=(r2,col), b, ch) with r = 2k+r2
            outv = out[:].rearrange(
                "b (k r2) c ch -> k (r2 c) b ch", r2=2
            )

            for g in range(GROUPS):
                # -- 1. gather: one contiguous RUN per window ---------------
                G = gpool.tile([128, RUN], F32, tag="G")
                nc.gpsimd.indirect_dma_start(
                    out=G[:],
                    out_offset=None,
                    in_=img[:],
                    in_offset=IndirectOffsetOnAxis(ap=idx_sb[:, g : g + 1], axis=0),
                )
                Gv = G[:].rearrange("p (r x) -> p r x", x=M)  # [128, 65, 128]

                fy1 = meta_sb[:, 4 * g + 0 : 4 * g + 1]  # 1 - fy
                fy = meta_sb[:, 4 * g + 1 : 4 * g + 2]
                fx1 = meta_sb[:, 4 * g + 2 : 4 * g + 3]  # 1 - fx
                fx = meta_sb[:, 4 * g + 3 : 4 * g + 4]

                # -- 2. vertical blend: t = (1-fy)*W[r] + fy*W[r+1] ---------
                u = uvpool.tile([128, N * ROWS], F32, tag="uv")
                uv = u[:].rearrange("p (r x) -> p r x", x=ROWS)
                nc.scalar.activation(uv, Gv[:, 1:ROWS, 0:ROWS], Copy, scale=fy)
                t = tpool.tile([128, N * ROWS], F32, tag="t")
                tv = t[:].rearrange("p (r x) -> p r x", x=ROWS)
                nc.vector.scalar_tensor_tensor(
                    tv, Gv[:, 0:N, 0:ROWS], fy1, uv, MULT, ADD
                )

                # -- horizontal blend: o = (1-fx)*t[col] + fx*t[col+1] ------
                v = uvpool.tile([128, N * N], F32, tag="uv")
                vv = v[:].rearrange("p (r x) -> p r x", x=N)
                nc.scalar.activation(vv, tv[:, :, 1:ROWS], Copy, scale=fx)
                o = opool.tile([128, N * N], F32, tag="o")
                ov = o[:].rearrange("p (r x) -> p r x", x=N)
                nc.vector.scalar_tensor_tensor(ov, tv[:, :, 0:N], fx1, vv, MULT, ADD)

                # -- 3. transpose 128-col chunks: partitions become (r2,col),
                #       free dim becomes windows (b*4+ch) -------------------
                Ob = obpool.tile([128, N * N], F32, tag="Ob")
                for q in range(8):
                    P4 = psump.tile([128, 512], F32, tag="P4")
                    for j in range(4):
                        k = q * 4 + j
                        nc.tensor.transpose(
                            P4[:, 128 * j : 128 * (j + 1)],
                            o[:, 128 * k : 128 * (k + 1)],
                            ident_sb[:],
                        )
                    dst = Ob[:, 512 * q : 512 * (q + 1)]
                    if q % 2 == 0:
                        nc.vector.tensor_copy(dst, P4[:])
                    else:
                        nc.scalar.activation(dst, P4[:], Copy)

                # -- 4. store: per chunk k (rows 2k, 2k+1 of all 32 samples)
                #    Ob[(r2,col), (k, beta, ch)] -> out[b, 2k+r2, col, ch]
                for k in range(32):
                    src = Ob[:, 128 * k : 128 * (k + 1)].rearrange(
                        "p (beta ch) -> p beta ch", ch=C
                    )
                    dst = outv[k, :, g * GSAMP : (g + 1) * GSAMP, :]
                    eng = nc.sync if k % 2 == 0 else nc.scalar
                    eng.dma_start(out=dst, in_=src)
    nc.finalize()
    return nc


def get_nc():
    if "nc" not in _NC_CACHE:
        _NC_CACHE["nc"] = _build_nc()
    return _NC_CACHE["nc"]


def make_core_inputs(padded_obj, positions):
    """Host-side prep: shard + window metadata. Returns list of in_maps."""
    padded_obj = np.asarray(padded_obj, dtype=np.float32)
    positions = np.asarray(positions, dtype=np.float32)
    ox = positions[:, 0, 0, :]  # [B, C] column offsets
    oy = positions[:, 0, 1, :]  # [B, C] row offsets
    c0 = np.float32((M - N) // 2)
    sx = (c0 + ox).astype(np.float32)
    sy = (c0 + oy).astype(np.float32)
    x0 = np.floor(sx).astype(np.int32)
    y0 = np.floor(sy).astype(np.int32)
    fx = (sx - x0.astype(np.float32)).astype(np.float32)
    fy = (sy - y0.astype(np.float32)).astype(np.float32)

    p = np.arange(128)
    brel = p // C  # 0..31 sample-within-group
    ch = p % C     # channel
    ident = np.eye(128, dtype=np.float32)

    in_maps = []
    for core in range(NCORES):
        s = slice(core * BC, (core + 1) * BC)
        img_c = np.ascontiguousarray(padded_obj[s, :, :, 0]).reshape(-1, 1)
        y0c, x0c = y0[s], x0[s]
        fyc, fxc = fy[s], fx[s]
        idx_c = np.empty((128, GROUPS), np.int32)
        meta_c = np.empty((128, 4 * GROUPS), np.float32)
        for g in range(GROUPS):
            bloc = g * GSAMP + brel
            idx_c[:, g] = (bloc * M + y0c[bloc, ch]) * M + x0c[bloc, ch]
            meta_c[:, 4 * g + 0] = np.float32(1.0) - fyc[bloc, ch]
            meta_c[:, 4 * g + 1] = fyc[bloc, ch]
            meta_c[:, 4 * g + 2] = np.float32(1.0) - fxc[bloc, ch]
            meta_c[:, 4 * g + 3] = fxc[bloc, ch]
        in_maps.append(
            {"img": img_c, "idx": idx_c, "meta": meta_c, "ident": ident}
        )
    return in_maps


def _make_runner(nc):
    """Build a persistent jitted SPMD executor for `nc` (compiles once).

    Mirrors concourse.bass2jax.run_bass_via_pjrt but caches the jitted
    function so repeated kernel() calls don't re-trigger neuronx-cc.
    """
    import jax
    from jax.sharding import Mesh, PartitionSpec
    from jax.experimental.shard_map import shard_map
    from concourse import bass2jax, mybir as mb

    bass2jax.install_neuronx_cc_hook()
    assert not nc.dbg_callbacks, "dbg callbacks unsupported under axon"

    extra_in_maps = {}
    if nc.dbg_addr is not None:
        extra_in_maps[nc.dbg_addr.name] = np.zeros((1, 2), np.uint32)
    partition_name = nc.partition_id_tensor.name if nc.partition_id_tensor else None

    in_names, out_names, out_avals = [], [], []
    for alloc in nc.m.functions[0].allocations:
        if not isinstance(alloc, mb.MemoryLocationSet):
            continue
        name = alloc.memorylocations[0].name
        if alloc.kind == "ExternalInput":
            if name != partition_name:
                in_names.append(name)
        elif alloc.kind == "ExternalOutput":
            out_names.append(name)
            out_avals.append(
                jax.core.ShapedArray(tuple(alloc.tensor_shape), mb.dt.np(alloc.dtype))
            )
    n_params = len(in_names)
    n_outs = len(out_avals)
    all_names = in_names + out_names
    if partition_name is not None:
        all_names = all_names + [partition_name]
    donate = tuple(range(n_params, n_params + n_outs))

    def _body(*args):
        operands = list(args)
        if partition_name is not None:
            operands.append(bass2jax.partition_id_tensor())
        outs = bass2jax._bass_exec_p.bind(
            *operands,
            out_avals=tuple(out_avals),
            in_names=tuple(all_names),
            out_names=tuple(out_names),
            lowering_input_output_aliases=(),
            sim_require_finite=True,
            sim_require_nnan=True,
            nc=nc,
        )
        return tuple(outs)

    devices = jax.devices()[:NCORES]
    mesh = Mesh(np.asarray(devices), ("core",))
    in_specs = (PartitionSpec("core"),) * (n_params + n_outs)
    out_specs = (PartitionSpec("core"),) * n_outs
    sharded = jax.jit(
        shard_map(_body, mesh=mesh, in_specs=in_specs, out_specs=out_specs,
                  check_rep=False),
        donate_argnums=donate,
        keep_unused=True,
    )

    def run(in_maps, device_only=False):
        if extra_in_maps:
            in_maps = [{**m, **extra_in_maps} for m in in_maps]
        concat_in = [
            np.concatenate([np.asarray(m[name]) for m in in_maps], axis=0)
            for name in in_names
        ]
        concat_zeros = [
            np.zeros((NCORES * a.shape[0], *a.shape[1:]), a.dtype) for a in out_avals
        ]
        out_arrs = sharded(*concat_in, *concat_zeros)
        if device_only:
            jax.block_until_ready(out_arrs)
            return None
        return {
            name: np.asarray(out_arrs[i]) for i, name in enumerate(out_names)
        }

    return run


def get_runner():
    if "run" not in _NC_CACHE:
        _NC_CACHE["run"] = _make_runner(get_nc())
    return _NC_CACHE["run"]


def kernel(padded_obj, positions, N=None):
    assert padded_obj.shape == (B, M, M, 1), padded_obj.shape
    in_maps = make_core_inputs(padded_obj, positions)
    out = get_runner()(in_maps)["out"]
    return np.ascontiguousarray(out).astype(np.float32)



# revision 3
# speedup vs baseline: 2.6435x; 2.6435x over previous
"""Bass/Trainium2 kernel for ExtractPatchesPosition (bilinear patch extraction).

Strategy (pure data parallel, batch sharded over 8 cores; 256 samples/core):

For each (sample b, channel c) the reference samples a translated N x N grid
out(r,col) = img(r + 32 + oy, col + 32 + ox) with bilinear interpolation.
With |offset| <= 20 and margin 32 the samples never leave the image, so the
whole patch is: take the (N+1) x (N+1) window at integer origin
(y0, x0) = (floor(32+oy), floor(32+ox)) and blend

    t = (1-fy)*W[r, x]   + fy*W[r+1, x]      (vertical 2-tap)
    o = (1-fx)*t[r, col] + fx*t[r, col+1]    (horizontal 2-tap)

Device pipeline, per group of 128 samples (partition = sample), one pass per
channel c (8 passes per core):
  1. one indirect DMA (SWDGE) gathers, per partition, a contiguous run of
     65*128 bf16 elements of the flat image starting at the window origin
     (s*128 + y0)*128 + x0.  Both data-dependent shifts are absorbed into the
     per-partition element-granularity start offset; inside the run the
     window sits at static offsets (r*128 + x).
  2. ACT (scale per partition) + DVE scalar_tensor_tensor do the two fused
     blends; the second blend writes channel-interleaved (stride 4) into a
     per-group staging tile ot[sample, (r, col, ch)].
  3. one HWDGE DMA per group stores ot -> out[g*128:(g+1)*128] as a single
     fully contiguous 4 MiB HBM write (32 KiB contiguous per sample).

The whole datapath runs in bf16 (rel-err budget is 2e-2; bf16 contributes
~5e-3), halving both gather and store HBM traffic vs f32.  The tiny
per-window metadata (int window origins, fractional weights) is precomputed
on host from `positions` (O(B*C) work) and passed as extra input tensors;
all O(B*N*N*C) data movement and math runs on device.
"""

import numpy as np

import concourse.bacc as bacc
import concourse.tile as tile
from concourse import mybir
from concourse.bass import IndirectOffsetOnAxis

B, M, N, C = 2048, 128, 64, 4
NCORES = 8
BC = B // NCORES          # 256 samples per core
P = 128                   # samples per group (one per partition)
GROUPS = BC // P          # 2 groups per core
PASSES = GROUPS * C       # 8 channel-passes per core
ROWS = N + 1              # 65 window rows
RUN = ROWS * M            # 8320 gathered elements per window
OUTW = N * N * C          # 16384 out elements per sample
F32 = mybir.dt.float32
BF16 = mybir.dt.bfloat16
Copy = mybir.ActivationFunctionType.Copy
MULT = mybir.AluOpType.mult
ADD = mybir.AluOpType.add

_NC_CACHE = {}


def _build_nc():
    nc = bacc.Bacc("TRN2")
    img = nc.declare_dram_parameter("img", [BC * M * M, 1], BF16, isOutput=False)
    idx = nc.declare_dram_parameter("idx", [128, PASSES], mybir.dt.int32, isOutput=False)
    meta = nc.declare_dram_parameter("meta", [128, 4 * PASSES], F32, isOutput=False)
    out = nc.declare_dram_parameter("out", [BC, OUTW], BF16, isOutput=True)

    with tile.TileContext(nc) as tc:
        with (
            tc.tile_pool(name="singles", bufs=1) as singles,
            tc.tile_pool(name="gpool", bufs=3) as gpool,
            tc.tile_pool(name="tpool", bufs=2) as tpool,
            tc.tile_pool(name="uvpool", bufs=2) as uvpool,
            tc.tile_pool(name="opool", bufs=2) as opool,
        ):
            idx_sb = singles.tile([128, PASSES], mybir.dt.int32)
            meta_sb = singles.tile([128, 4 * PASSES], F32)
            nc.sync.dma_start(idx_sb[:], idx[:])
            nc.sync.dma_start(meta_sb[:], meta[:])

            for g in range(GROUPS):
                ot = opool.tile([128, OUTW], BF16, tag="ot")
                # view [p, r, col, ch]; channel c writes the stride-4 slice
                otv = ot[:].rearrange("p (r c ch) -> p r c ch", c=N, ch=C)
                for c in range(C):
                    ps = g * C + c
                    # -- 1. gather: one contiguous RUN per window -----------
                    G = gpool.tile([128, RUN], BF16, tag="G")
                    nc.gpsimd.indirect_dma_start(
                        out=G[:],
                        out_offset=None,
                        in_=img[:],
                        in_offset=IndirectOffsetOnAxis(
                            ap=idx_sb[:, ps : ps + 1], axis=0
                        ),
                    )
                    Gv = G[:].rearrange("p (r x) -> p r x", x=M)  # [128, 65, 128]

                    fy1 = meta_sb[:, 4 * ps + 0 : 4 * ps + 1]  # 1 - fy
                    fy = meta_sb[:, 4 * ps + 1 : 4 * ps + 2]
                    fx1 = meta_sb[:, 4 * ps + 2 : 4 * ps + 3]  # 1 - fx
                    fx = meta_sb[:, 4 * ps + 3 : 4 * ps + 4]

                    # -- 2. vertical blend: t = (1-fy)*W[r] + fy*W[r+1] -----
                    u = uvpool.tile([128, N * ROWS], BF16, tag="uv")
                    uv = u[:].rearrange("p (r x) -> p r x", x=ROWS)
                    nc.scalar.activation(uv, Gv[:, 1:ROWS, 0:ROWS], Copy, scale=fy)
                    t = tpool.tile([128, N * ROWS], BF16, tag="t")
                    tv = t[:].rearrange("p (r x) -> p r x", x=ROWS)
                    nc.vector.scalar_tensor_tensor(
                        tv, Gv[:, 0:N, 0:ROWS], fy1, uv, MULT, ADD
                    )

                    # -- horizontal blend: o = (1-fx)*t[col] + fx*t[col+1],
                    #    written channel-interleaved into the staging tile --
                    v = uvpool.tile([128, N * N], BF16, tag="uv")
                    vv = v[:].rearrange("p (r x) -> p r x", x=N)
                    nc.scalar.activation(vv, tv[:, :, 1:ROWS], Copy, scale=fx)
                    nc.vector.scalar_tensor_tensor(
                        otv[:, :, :, c], tv[:, :, 0:N], fx1, vv, MULT, ADD
                    )

                # -- 3. store: one fully contiguous 4 MiB write -------------
                nc.sync.dma_start(out=out[g * P : (g + 1) * P, :], in_=ot[:])
    nc.finalize()
    return nc


def get_nc():
    if "nc" not in _NC_CACHE:
        _NC_CACHE["nc"] = _build_nc()
    return _NC_CACHE["nc"]


def make_core_inputs(padded_obj, positions):
    """Host-side prep: shard + window metadata. Returns list of in_maps."""
    import ml_dtypes

    padded_obj = np.asarray(padded_obj, dtype=np.float32)
    positions = np.asarray(positions, dtype=np.float32)
    ox = positions[:, 0, 0, :]  # [B, C] column offsets
    oy = positions[:, 0, 1, :]  # [B, C] row offsets
    c0 = np.float32((M - N) // 2)
    sx = (c0 + ox).astype(np.float32)
    sy = (c0 + oy).astype(np.float32)
    x0 = np.floor(sx).astype(np.int32)
    y0 = np.floor(sy).astype(np.int32)
    fx = (sx - x0.astype(np.float32)).astype(np.float32)
    fy = (sy - y0.astype(np.float32)).astype(np.float32)

    img_bf = padded_obj[:, :, :, 0].astype(ml_dtypes.bfloat16)

    in_maps = []
    for core in range(NCORES):
        s = slice(core * BC, (core + 1) * BC)
        img_c = np.ascontiguousarray(img_bf[s]).reshape(-1, 1)
        y0c, x0c = y0[s], x0[s]
        fyc, fxc = fy[s], fx[s]
        idx_c = np.empty((128, PASSES), np.int32)
        meta_c = np.empty((128, 4 * PASSES), np.float32)
        p = np.arange(128)
        for g in range(GROUPS):
            sloc = g * P + p
            for c in range(C):
                ps = g * C + c
                idx_c[:, ps] = (sloc * M + y0c[sloc, c]) * M + x0c[sloc, c]
                meta_c[:, 4 * ps + 0] = np.float32(1.0) - fyc[sloc, c]
                meta_c[:, 4 * ps + 1] = fyc[sloc, c]
                meta_c[:, 4 * ps + 2] = np.float32(1.0) - fxc[sloc, c]
                meta_c[:, 4 * ps + 3] = fxc[sloc, c]
        in_maps.append({"img": img_c, "idx": idx_c, "meta": meta_c})
    return in_maps


def _make_runner(nc):
    """Build a persistent jitted SPMD executor for `nc` (compiles once).

    Mirrors concourse.bass2jax.run_bass_via_pjrt but caches the jitted
    function so repeated kernel() calls don't re-trigger neuronx-cc.
    """
    import jax
    from jax.sharding import Mesh, PartitionSpec
    from jax.experimental.shard_map import shard_map
    from concourse import bass2jax, mybir as mb

    bass2jax.install_neuronx_cc_hook()
    assert not nc.dbg_callbacks, "dbg callbacks unsupported under axon"

    extra_in_maps = {}
    if nc.dbg_addr is not None:
        extra_in_maps[nc.dbg_addr.name] = np.zeros((1, 2), np.uint32)
    partition_name = nc.partition_id_tensor.name if nc.partition_id_tensor else None

    in_names, out_names, out_avals = [], [], []
    for alloc in nc.m.functions[0].allocations:
        if not isinstance(alloc, mb.MemoryLocationSet):
            continue
        name = alloc.memorylocations[0].name
        if alloc.kind == "ExternalInput":
            if name != partition_name:
                in_names.append(name)
        elif alloc.kind == "ExternalOutput":
            out_names.append(name)
            out_avals.append(
                jax.core.ShapedArray(tuple(alloc.tensor_shape), mb.dt.np(alloc.dtype))
            )
    n_params = len(in_names)
    n_outs = len(out_avals)
    all_names = in_names + out_names
    if partition_name is not None:
        all_names = all_names + [partition_name]
    donate = tuple(range(n_params, n_params + n_outs))

    def _body(*args):
        operands = list(args)
        if partition_name is not None:
            operands.append(bass2jax.partition_id_tensor())
        outs = bass2jax._bass_exec_p.bind(
            *operands,
            out_avals=tuple(out_avals),
            in_names=tuple(all_names),
            out_names=tuple(out_names),
            lowering_input_output_aliases=(),
            sim_require_finite=True,
            sim_require_nnan=True,
            nc=nc,
        )
        return tuple(outs)

    devices = jax.devices()[:NCORES]
    mesh = Mesh(np.asarray(devices), ("core",))
    in_specs = (PartitionSpec("core"),) * (n_params + n_outs)
    out_specs = (PartitionSpec("core"),) * n_outs
    sharded = jax.jit(
        shard_map(_body, mesh=mesh, in_specs=in_specs, out_specs=out_specs,
                  check_rep=False),
        donate_argnums=donate,
        keep_unused=True,
    )

    def run(in_maps, device_only=False):
        if extra_in_maps:
            in_maps = [{**m, **extra_in_maps} for m in in_maps]
        concat_in = [
            np.concatenate([np.asarray(m[name]) for m in in_maps], axis=0)
            for name in in_names
        ]
        concat_zeros = [
            np.zeros((NCORES * a.shape[0], *a.shape[1:]), a.dtype) for a in out_avals
        ]
        out_arrs = sharded(*concat_in, *concat_zeros)
        if device_only:
            jax.block_until_ready(out_arrs)
            return None
        return {
            name: np.asarray(out_arrs[i]) for i, name in enumerate(out_names)
        }

    return run


def get_runner():
    if "run" not in _NC_CACHE:
        _NC_CACHE["run"] = _make_runner(get_nc())
    return _NC_CACHE["run"]


def kernel(padded_obj, positions, N=None):
    assert padded_obj.shape == (B, M, M, 1), padded_obj.shape
    in_maps = make_core_inputs(padded_obj, positions)
    out = get_runner()(in_maps)["out"]
    return np.ascontiguousarray(out.astype(np.float32).reshape(B, 64, 64, C))
